# revision 1
# baseline (speedup 1.0000x reference)
"""NMS detection decoder (nn_DecoderV1) Bass/Tile kernel for 8x TRN2 NeuronCores.

Strategy (data parallel, 2 images per core):
  - DMA only the score channel (1/5 of input bytes) as [128, 3200] per image.
  - DVE max8 -> per-row top-8 values V1 [128,8]; max_index -> in-row positions.
    (Progressive duplicate matching in HW gives ascending positions for ties,
     matching jax.lax.top_k's lower-index-first tie-break within a row.)
  - Prune threshold t0 = ~100th largest row-max (guarantees >=100 candidates
    above it); mask V1 -> candidate set (~280 slots), compacted to a fixed
    [1, 512] row via gpsimd sparse_gather (tail filled with -1).
  - Rank pass 1 (ACT engine): for each of the 8 slot columns, accumulate
    sum(sign(B - v)) over the candidate row -> rho_hat = gt + eq_others/2.
  - Tie-broken key KEY2 = rho_hat*2048 + q (q = 8*row + col + 16, exact in
    fp32; q-order == flat-index order, matching jax tie-break across rows).
  - Rank pass 2 over compacted KEY2 row -> exact final rank (a permutation
    over candidates); tail/tie corrections use nf2 (sparse_gather count).
  - Resolution: EQ[slot, k] = [rank==k] (k=0..99), then PE matmuls
    sum_slots EQ * {value, flat_index, 1} -> sorted top-100 values + indices
    + count check, in [100, 1] partition layout.
  - Boxes for the top-100 gathered by indirect DMA (4 fp32 each) from HBM.
  - IoU suppressor matrix via PE transpose + broadcasts; suppress test uses
    the division-free form (2*inter > union) & (union > 0) which matches
    inter/union > 0.5 except on measure-zero boundaries.
  - Greedy NMS as a fixed-point iteration k <- vmask & (S^T k == 0) on the
    PE (converges in <= 3 iters on this distribution) + convergence check.
  - Self-verification flags (capture depth, count check, sparse overflow,
    NMS convergence) -> host falls back to an exact numpy path per flagged
    image (probability ~0; pure safety net).

Engine notes: gpsimd keeps the sparse_gather library loaded for the whole
kernel, so every other gpsimd op used here must be a builtin (tensor_scalar /
scalar_tensor_tensor / memset / copy / affine_select); tensor_tensor forms are
expressed as scalar_tensor_tensor with op0=bypass. Cross-partition reductions
and broadcasts go through the PE (transpose / ones-matmul).
"""

import os
import sys

import numpy as np

for _p in ("/opt/trn_rl_repo",):
    if _p not in sys.path and os.path.isdir(_p):
        sys.path.insert(0, _p)

import concourse.bacc as bacc
import concourse.mybir as mybir
from concourse.bass import AP, IndirectOffsetOnAxis
from concourse.masks import make_identity
from concourse.tile import TileContext

P = 128
F = 3200
N = P * F  # 409600 spatial positions per image
NIMG = 2   # images per core
K = 100
W = 512    # compacted candidate row width (16 partitions x 32)
T_NMS = 5  # fixed-point iterations (measured worst case 3) + 1 for the check
NEG = -3.0e38
BIGKEY = 8.0e6  # sentinel key for non-candidates (> max real key ~2.1e6)
f32 = mybir.dt.float32
bf16 = mybir.dt.bfloat16
u32 = mybir.dt.uint32
i32 = mybir.dt.int32
Alu = mybir.AluOpType
Act = mybir.ActivationFunctionType


def _ap3(t, c0, c1, s0, s1):
    """Build a [P, c0, c1] AP over SBUF tile t with free steps (s0, s1)."""
    base = t[:]
    return AP(base.tensor, base.offset, [base.ap[0], [s0, c0], [s1, c1]])


def _col(t, j):
    return t[:, j:j + 1]


def build_nc():
    nc = bacc.Bacc()
    preds = nc.dram_tensor("preds", [NIMG, 5, N], f32, kind="ExternalInput")
    out = nc.dram_tensor("out", [NIMG, K, 5], f32, kind="ExternalOutput")
    flags = nc.dram_tensor("flags", [NIMG, 8], f32, kind="ExternalOutput")

    with TileContext(nc) as tc:
        with (
            tc.tile_pool(name="const", bufs=1) as cpool,
            tc.tile_pool(name="sb", bufs=2) as pool,
            tc.tile_pool(name="big", bufs=2) as bigpool,
            tc.tile_pool(name="psBB", bufs=2, space="PSUM") as psBB,
            tc.tile_pool(name="psR", bufs=2, space="PSUM") as psR,
            tc.tile_pool(name="psS", bufs=2, space="PSUM") as psS,
            tc.tile_pool(name="psRB", bufs=1, space="PSUM") as psRB,
            tc.tile_pool(name="psup", bufs=1, space="PSUM") as psup,
        ):
            # ---------------- shared constants ----------------
            ident = cpool.tile([P, P], f32)
            make_identity(nc, ident[:])
            ones_r = cpool.tile([1, P], f32)
            nc.vector.memset(ones_r[:], 1.0)
            ones_col = cpool.tile([P, 1], f32)
            nc.vector.memset(ones_col[:], 1.0)

            qgrid_i = cpool.tile([P, 8], i32)
            nc.gpsimd.iota(qgrid_i[:], pattern=[[1, 8]], base=16,
                           channel_multiplier=8)
            qgridf = cpool.tile([P, 8], f32)
            nc.vector.tensor_copy(qgridf[:], qgrid_i[:])

            rowb_i = cpool.tile([P, 1], i32)
            nc.gpsimd.iota(rowb_i[:], pattern=[[0, 1]], channel_multiplier=F)
            rowbase = cpool.tile([P, 1], f32)
            nc.vector.tensor_copy(rowbase[:], rowb_i[:])

            k100_i = cpool.tile([P, K], i32)
            nc.gpsimd.iota(k100_i[:], pattern=[[1, K]], channel_multiplier=0)
            k100f = cpool.tile([P, K], f32)
            nc.vector.tensor_copy(k100f[:], k100_i[:])

            # wrap-position grid for sparse_gather tail masking:
            # stream index s = f*16 + pp at (partition pp, free f)
            wrap_i = cpool.tile([16, 32], i32)
            nc.gpsimd.iota(wrap_i[:], pattern=[[16, 32]], channel_multiplier=1)
            wrapf = cpool.tile([16, 32], f32)
            nc.vector.tensor_copy(wrapf[:], wrap_i[:])
            neg1t = cpool.tile([16, 32], f32)
            nc.vector.memset(neg1t[:], -1.0)

            # row-selector matrices: sel5[j][k, p] = [k == j] (PE row-broadcast)
            ones5 = cpool.tile([5, K], f32)
            nc.vector.memset(ones5[:], 1.0)
            sel5 = cpool.tile([5, 5 * K], f32)
            for j in range(5):
                nc.gpsimd.affine_select(
                    out=sel5[:, j * K:(j + 1) * K], in_=ones5[:],
                    pattern=[[0, K]], compare_op=Alu.is_equal, fill=0.0,
                    base=-j, channel_multiplier=1)

            for b in range(NIMG):
                # ---------------- load scores ----------------
                sc = bigpool.tile([P, F], f32, tag="sc")
                src = preds[b, 0].rearrange("(p f) -> p f", p=P)
                for j in range(4):
                    nc.sync.dma_start(
                        out=sc[j * 32:(j + 1) * 32, :],
                        in_=src[j * 32:(j + 1) * 32, :])

                # ---------------- per-row top-8 ----------------
                V1 = pool.tile([P, 8], f32, tag="V1")
                nc.vector.max(out=V1[:], in_=sc[:])
                nV1 = pool.tile([P, 8], f32, tag="nV1")
                nc.gpsimd.tensor_scalar_mul(nV1[:], V1[:], -1.0)
                I1 = pool.tile([P, 8], u32, tag="I1")
                nc.vector.max_index(out=I1[:], in_max=V1[:], in_values=sc[:])

                # ---------------- t0 ~ 100th largest row-max ----------------
                c0T = psS.tile([1, P], f32, tag="psmall")
                nc.tensor.transpose(out=c0T[:], in_=_col(V1, 0),
                                    identity=ident[:])
                c0r = pool.tile([1, P], f32, tag="c0r")
                nc.scalar.activation(c0r[:], c0T[:], Act.Copy)
                bb0 = psBB.tile([P, P], f32, tag="BB")
                nc.tensor.matmul(out=bb0[:], lhsT=ones_r[:], rhs=c0r[:],
                                 start=True, stop=True)
                tr0 = pool.tile([P, P], bf16, tag="tr0")
                sg0 = pool.tile([P, 1], f32, tag="sg0")
                nc.scalar.activation(tr0[:], bb0[:], Act.Sign,
                                     bias=_col(nV1, 0), scale=1.0,
                                     accum_out=sg0[:])
                # rho0 = gt + eqo/2 among the 128 row-maxes
                rho0 = pool.tile([P, 1], f32, tag="rho0")
                nc.gpsimd.tensor_scalar(out=rho0[:], in0=sg0[:], scalar1=127.0,
                                        scalar2=0.5, op0=Alu.add, op1=Alu.mult)
                m0 = pool.tile([P, 1], u32, tag="m0")
                nc.gpsimd.tensor_scalar(out=m0[:], in0=rho0[:], scalar1=101.0,
                                        scalar2=None, op0=Alu.is_le)
                # -t0 = max over masked(-rowmax); min over ~100 largest rowmaxes
                nmt0 = pool.tile([P, 1], f32, tag="nmt0")
                nc.vector.memset(nmt0[:], NEG)
                nc.vector.copy_predicated(nmt0[:], m0[:], _col(nV1, 0))
                nmt0T = psS.tile([1, P], f32, tag="psmall")
                nc.tensor.transpose(out=nmt0T[:], in_=nmt0[:], identity=ident[:])
                nmt0r = pool.tile([1, P], f32, tag="nmt0r")
                nc.scalar.activation(nmt0r[:], nmt0T[:], Act.Copy)
                nt0s = pool.tile([1, 1], f32, tag="nt0s")
                nc.vector.reduce_max(out=nt0s[:], in_=nmt0r[:],
                                     axis=mybir.AxisListType.X)
                nt0ps = psS.tile([P, 1], f32, tag="psmall")
                nc.tensor.matmul(out=nt0ps[:], lhsT=ones_r[:], rhs=nt0s[:],
                                 start=True, stop=True)
                nt0 = pool.tile([P, 1], f32, tag="nt0")
                nc.scalar.activation(nt0[:], nt0ps[:], Act.Copy)

                # ---------------- candidate mask + sentinels ----------------
                maskc = pool.tile([P, 8], u32, tag="maskc")
                nc.gpsimd.tensor_scalar(out=maskc[:], in0=nV1[:],
                                        scalar1=nt0[:, 0:1],
                                        scalar2=None, op0=Alu.is_le)
                vs = pool.tile([P, 8], f32, tag="vs")
                nc.gpsimd.memset(vs[:], -1.0)
                nc.vector.copy_predicated(vs[:], maskc[:], V1[:])

                # wrap [128,8] -> [16,64] (natural partition fold), compact,
                # unwrap [16,32] -> [1,512] (partition-major concat)
                vs16 = pool.tile([16, 64], f32, tag="vs16")
                nc.sync.dma_start(out=vs16[:], in_=vs[:])
                cmp1 = pool.tile([16, 32], f32, tag="cmp1")
                nf1 = pool.tile([1, 1], u32, tag="nf1")
                nc.gpsimd.sparse_gather(out=cmp1[:], in_=vs16[:],
                                        num_found=nf1[:])
                # HW sparse_gather leaves garbage past num_found; force -1
                nf1f = pool.tile([1, 1], f32, tag="nf1f")
                nc.gpsimd.tensor_copy(nf1f[:], nf1[:])
                nps1 = psS.tile([16, 1], f32, tag="psmall")
                nc.tensor.matmul(out=nps1[:], lhsT=ones_r[:, 0:16], rhs=nf1f[:],
                                 start=True, stop=True)
                nfb1 = pool.tile([16, 1], f32, tag="nfb1")
                nc.scalar.activation(nfb1[:], nps1[:], Act.Copy)
                tailm1 = pool.tile([16, 32], u32, tag="tailm1")
                nc.gpsimd.tensor_scalar(out=tailm1[:], in0=wrapf[:],
                                        scalar1=nfb1[:, 0:1], scalar2=None,
                                        op0=Alu.is_ge)
                nc.vector.copy_predicated(cmp1[:], tailm1[:], neg1t[:])
                B1 = pool.tile([1, W], f32, tag="B1")
                nc.sync.dma_start(
                    out=B1[:].rearrange("a (pp f) -> a pp f", pp=16),
                    in_=cmp1[:])
                BB1 = psBB.tile([P, W], f32, tag="BB")
                nc.tensor.matmul(out=BB1[:], lhsT=ones_r[:], rhs=B1[:],
                                 start=True, stop=True)

                # ---------------- rank pass 1 (ACT sign-accum) ----------------
                # tail(-1) counts as "lt" for candidate queries =>
                # rho_hat = (sig + 511)/2 independent of nf1.
                trA = bigpool.tile([P, W], bf16, tag="trA")
                SG1 = pool.tile([P, 8], f32, tag="SG1")
                for c in range(8):
                    nc.scalar.activation(trA[:], BB1[:], Act.Sign,
                                         bias=_col(nV1, c), scale=1.0,
                                         accum_out=_col(SG1, c))
                rho = pool.tile([P, 8], f32, tag="rho")
                nc.gpsimd.tensor_scalar(out=rho[:], in0=SG1[:], scalar1=511.0,
                                        scalar2=1024.0, op0=Alu.add,
                                        op1=Alu.mult)  # rho_hat * 2048
                KEY2 = pool.tile([P, 8], f32, tag="KEY2")
                nc.vector.tensor_tensor(out=KEY2[:], in0=rho[:], in1=qgridf[:],
                                        op=Alu.add)
                KEY2m = pool.tile([P, 8], f32, tag="KEY2m")
                nc.gpsimd.memset(KEY2m[:], BIGKEY)
                nc.vector.copy_predicated(KEY2m[:], maskc[:], KEY2[:])
                nKEY2m = pool.tile([P, 8], f32, tag="nKEY2m")
                nc.gpsimd.tensor_scalar_mul(nKEY2m[:], KEY2m[:], -1.0)
                vs2 = pool.tile([P, 8], f32, tag="vs2")
                nc.gpsimd.memset(vs2[:], -1.0)
                nc.vector.copy_predicated(vs2[:], maskc[:], KEY2[:])

                vs216 = pool.tile([16, 64], f32, tag="vs216")
                nc.sync.dma_start(out=vs216[:], in_=vs2[:])
                cmp2 = pool.tile([16, 32], f32, tag="cmp2")
                nf2 = pool.tile([1, 1], u32, tag="nf2")
                nc.gpsimd.sparse_gather(out=cmp2[:], in_=vs216[:],
                                        num_found=nf2[:])
                nf2f = pool.tile([1, 1], f32, tag="nf2f")
                nc.gpsimd.tensor_copy(nf2f[:], nf2[:])
                nps2 = psS.tile([16, 1], f32, tag="psmall")
                nc.tensor.matmul(out=nps2[:], lhsT=ones_r[:, 0:16], rhs=nf2f[:],
                                 start=True, stop=True)
                nfb2 = pool.tile([16, 1], f32, tag="nfb2")
                nc.scalar.activation(nfb2[:], nps2[:], Act.Copy)
                tailm2 = pool.tile([16, 32], u32, tag="tailm2")
                nc.gpsimd.tensor_scalar(out=tailm2[:], in0=wrapf[:],
                                        scalar1=nfb2[:, 0:1], scalar2=None,
                                        op0=Alu.is_ge)
                nc.vector.copy_predicated(cmp2[:], tailm2[:], neg1t[:])
                B2 = pool.tile([1, W], f32, tag="B2")
                nc.sync.dma_start(
                    out=B2[:].rearrange("a (pp f) -> a pp f", pp=16),
                    in_=cmp2[:])
                BB2 = psBB.tile([P, W], f32, tag="BB")
                nc.tensor.matmul(out=BB2[:], lhsT=ones_r[:], rhs=B2[:],
                                 start=True, stop=True)

                # nf2 broadcast to all partitions (via PE) for tail corrections
                nfbps = psS.tile([P, 1], f32, tag="psmall")
                nc.tensor.matmul(out=nfbps[:], lhsT=ones_r[:], rhs=nf2f[:],
                                 start=True, stop=True)
                nfb = pool.tile([P, 1], f32, tag="nfb")
                nc.scalar.activation(nfb[:], nfbps[:], Act.Copy)
                nfm1 = pool.tile([P, 1], f32, tag="nfm1")  # nf2 - 1
                nc.gpsimd.tensor_scalar(out=nfm1[:], in0=nfb[:], scalar1=1.0,
                                        scalar2=None, op0=Alu.subtract)
                nfmW = pool.tile([P, 1], f32, tag="nfmW")  # nf2 - W
                nc.gpsimd.tensor_scalar(out=nfmW[:], in0=nfb[:],
                                        scalar1=float(W),
                                        scalar2=None, op0=Alu.subtract)

                # ---------------- rank pass 2 (tie-broken final ranks) -------
                RNK = pool.tile([P, 8], f32, tag="RNK")
                SG2 = pool.tile([P, 8], f32, tag="SG2")
                trB = bigpool.tile([P, W], bf16, tag="trB")
                for c in range(8):
                    nc.scalar.activation(trB[:], BB2[:], Act.Sign,
                                         bias=_col(nKEY2m, c), scale=1.0,
                                         accum_out=_col(SG2, c))
                # sig = 2*gt - 511 => gt = (sig+511)/2 ; final = (nf2-1) - gt
                gt2 = pool.tile([P, 8], f32, tag="gt2")
                nc.gpsimd.tensor_scalar(out=gt2[:], in0=SG2[:], scalar1=511.0,
                                        scalar2=0.5, op0=Alu.add, op1=Alu.mult)
                nc.gpsimd.tensor_scalar(out=RNK[:], in0=gt2[:],
                                        scalar1=nfm1[:, 0:1], scalar2=-1.0,
                                        op0=Alu.subtract, op1=Alu.mult)

                # ---------------- resolution ----------------
                EQ = bigpool.tile([P, 8 * K], f32, tag="EQ")
                nc.vector.tensor_tensor(
                    out=_ap3(EQ, 8, K, K, 1),
                    in0=_ap3(RNK, 8, K, 1, 0),
                    in1=_ap3(k100f, 8, K, 0, 1),
                    op=Alu.is_equal)

                I1f = pool.tile([P, 8], f32, tag="I1f")
                nc.gpsimd.tensor_copy(I1f[:], I1[:])
                gfx = pool.tile([P, 8], f32, tag="gfx")
                nc.gpsimd.tensor_scalar(out=gfx[:], in0=I1f[:],
                                        scalar1=rowbase[:, 0:1],
                                        scalar2=None, op0=Alu.add)
                rhs3 = pool.tile([P, 24], f32, tag="rhs3")
                nc.gpsimd.tensor_copy(
                    AP(rhs3[:].tensor, rhs3[:].offset, [rhs3[:].ap[0], [3, 8]]),
                    V1[:])
                nc.gpsimd.tensor_copy(
                    AP(rhs3[:].tensor, rhs3[:].offset + 1,
                       [rhs3[:].ap[0], [3, 8]]), gfx[:])
                nc.gpsimd.memset(
                    AP(rhs3[:].tensor, rhs3[:].offset + 2,
                       [rhs3[:].ap[0], [3, 8]]), 1.0)

                Rps = psR.tile([K, 3], f32, tag="Rps")
                for c in range(8):
                    nc.tensor.matmul(out=Rps[:], lhsT=EQ[:, c * K:(c + 1) * K],
                                     rhs=rhs3[:, 3 * c:3 * c + 3],
                                     start=(c == 0), stop=(c == 7))
                Rsb = pool.tile([K, 3], f32, tag="Rsb")
                nc.scalar.activation(Rsb[:], Rps[:], Act.Copy)
                idxu = pool.tile([K, 1], u32, tag="idxu")
                nc.gpsimd.tensor_copy(idxu[:], Rsb[:, 1:2])

                # ---------------- gather boxes ----------------
                bx = pool.tile([K, 4], f32, tag="bx")
                flat = preds[:].rearrange("a b (c d) -> (a b c) d", d=1)
                for c in range(4):
                    nc.gpsimd.indirect_dma_start(
                        out=_col(bx, c), out_offset=None,
                        in_=flat,
                        in_offset=IndirectOffsetOnAxis(ap=idxu[:, 0:1], axis=0),
                        element_offset=(b * 5 + 1 + c) * N,
                        bounds_check=N - 1, oob_is_err=False)

                # ---------------- IoU suppressor matrix ----------------
                w0 = pool.tile([K, 1], f32, tag="w0")
                nc.vector.tensor_tensor(out=w0[:], in0=_col(bx, 2),
                                        in1=_col(bx, 0), op=Alu.subtract)
                h0 = pool.tile([K, 1], f32, tag="h0")
                nc.vector.tensor_tensor(out=h0[:], in0=_col(bx, 3),
                                        in1=_col(bx, 1), op=Alu.subtract)
                pk5 = pool.tile([K, 5], f32, tag="pk5")
                nc.gpsimd.tensor_copy(pk5[:, 0:4], bx[:])
                nc.vector.tensor_tensor(out=pk5[:, 4:5], in0=w0[:], in1=h0[:],
                                        op=Alu.mult)
                T5 = psS.tile([5, K], f32, tag="psmall")
                nc.tensor.transpose(out=T5[:], in_=pk5[:],
                                    identity=ident[0:K, 0:K])
                T5sb = pool.tile([5, K], f32, tag="T5sb")
                nc.scalar.activation(T5sb[:], T5[:], Act.Copy)
                RB = psRB.tile([K, 5 * K], f32, tag="RB")
                for j in range(5):
                    nc.tensor.matmul(out=RB[:, j * K:(j + 1) * K],
                                     lhsT=sel5[:, j * K:(j + 1) * K],
                                     rhs=T5sb[:], start=True, stop=True)
                RBs = bigpool.tile([K, 5 * K], f32, tag="RBs")
                nc.scalar.activation(RBs[:], RB[:], Act.Copy)
                x1r, y1r, x2r, y2r, ar = (
                    RBs[:, j * K:(j + 1) * K] for j in range(5))

                # wh tile: [:, 0:K] = w, [:, K:2K] = h (single relu ts op)
                wh = pool.tile([K, 2 * K], f32, tag="wh")
                xx1 = pool.tile([K, K], f32, tag="xx1")
                nc.vector.tensor_tensor(out=xx1[:],
                                        in0=_col(bx, 0).to_broadcast([K, K]),
                                        in1=x1r, op=Alu.max)
                xx2 = pool.tile([K, K], f32, tag="xx2")
                nc.vector.tensor_tensor(out=xx2[:],
                                        in0=_col(bx, 2).to_broadcast([K, K]),
                                        in1=x2r, op=Alu.min)
                nc.vector.tensor_tensor(out=wh[:, 0:K], in0=xx2[:], in1=xx1[:],
                                        op=Alu.subtract)
                yy1 = pool.tile([K, K], f32, tag="yy1")
                nc.vector.tensor_tensor(out=yy1[:],
                                        in0=_col(bx, 1).to_broadcast([K, K]),
                                        in1=y1r, op=Alu.max)
                yy2 = pool.tile([K, K], f32, tag="yy2")
                nc.vector.tensor_tensor(out=yy2[:],
                                        in0=_col(bx, 3).to_broadcast([K, K]),
                                        in1=y2r, op=Alu.min)
                nc.vector.tensor_tensor(out=wh[:, K:2 * K], in0=yy2[:],
                                        in1=yy1[:], op=Alu.subtract)
                nc.vector.tensor_scalar_max(wh[:], wh[:], 0.0)
                inter = pool.tile([K, K], f32, tag="inter")
                nc.vector.tensor_tensor(out=inter[:], in0=wh[:, 0:K],
                                        in1=wh[:, K:2 * K], op=Alu.mult)
                un = pool.tile([K, K], f32, tag="un")
                nc.vector.scalar_tensor_tensor(out=un[:], in0=ar,
                                               scalar=pk5[:, 4:5], in1=inter[:],
                                               op0=Alu.add, op1=Alu.subtract)
                gt1 = pool.tile([K, K], f32, tag="gt1")
                nc.vector.scalar_tensor_tensor(out=gt1[:], in0=inter[:],
                                               scalar=2.0, in1=un[:],
                                               op0=Alu.mult, op1=Alu.is_gt)
                M = pool.tile([K, K], f32, tag="M")
                nc.vector.scalar_tensor_tensor(out=M[:], in0=un[:], scalar=0.0,
                                               in1=gt1[:], op0=Alu.is_gt,
                                               op1=Alu.mult)
                S = pool.tile([K, K], f32, tag="S")
                nc.gpsimd.affine_select(out=S[:], in_=M[:], pattern=[[1, K]],
                                        compare_op=Alu.is_gt, fill=0.0,
                                        base=0, channel_multiplier=-1)

                # ---------------- greedy NMS fixed point ----------------
                vmask = pool.tile([K, 1], f32, tag="vmask")
                nc.gpsimd.tensor_scalar(out=vmask[:], in0=Rsb[:, 0:1],
                                        scalar1=0.0, scalar2=None,
                                        op0=Alu.is_gt)
                kbufs = [
                    pool.tile([K, 1], f32, tag=f"kb{i}", name=f"kb{i}_{b}")
                    for i in range(3)
                ]
                nc.gpsimd.tensor_copy(kbufs[0][:], vmask[:])
                kcur = kbufs[0]
                for t in range(T_NMS):
                    sup = psup.tile([K, 1], f32, tag="sup")
                    nc.tensor.matmul(out=sup[:], lhsT=S[:], rhs=kcur[:],
                                     start=True, stop=True)
                    dst = kbufs[2] if t == T_NMS - 1 else \
                        kbufs[1 - (t % 2)] if t % 2 == 0 else kbufs[t % 2]
                    dst = kbufs[(t + 1) % 2] if t < T_NMS - 1 else kbufs[2]
                    nc.vector.scalar_tensor_tensor(out=dst[:], in0=sup[:],
                                                   scalar=0.0, in1=vmask[:],
                                                   op0=Alu.is_equal,
                                                   op1=Alu.mult)
                    kprev, kcur = kcur, dst
                # convergence check: sum((k_T - k_{T-1})^2) via PE
                cd2 = pool.tile([K, 1], f32, tag="cd2")
                nc.vector.tensor_tensor(out=cd2[:], in0=kcur[:], in1=kprev[:],
                                        op=Alu.not_equal)
                fconv = psS.tile([1, 1], f32, tag="psmall")
                nc.tensor.matmul(out=fconv[:], lhsT=cd2[:],
                                 rhs=ones_col[0:K, :], start=True, stop=True)

                # ---------------- outputs ----------------
                out5 = pool.tile([K, 5], f32, tag="out5")
                nc.vector.tensor_tensor(out=out5[:, 0:1], in0=Rsb[:, 0:1],
                                        in1=kcur[:], op=Alu.mult)
                nc.vector.tensor_tensor(out=out5[:, 1:5], in0=bx[:],
                                        in1=kcur[:].to_broadcast([K, 4]),
                                        op=Alu.mult)
                nc.sync.dma_start(out=out[b], in_=out5[:])

                # ---------------- flags ----------------
                fcap = pool.tile([P, 1], f32, tag="fcap")
                nc.gpsimd.tensor_scalar(out=fcap[:], in0=RNK[:, 7:8],
                                        scalar1=99.0, scalar2=None,
                                        op0=Alu.is_le)
                fcaps = psS.tile([1, 1], f32, tag="psmall")
                nc.tensor.matmul(out=fcaps[:], lhsT=fcap[:], rhs=ones_col[:],
                                 start=True, stop=True)
                ce2 = pool.tile([K, 1], f32, tag="ce2")
                nc.vector.tensor_scalar(out=ce2[:], in0=Rsb[:, 2:3],
                                        scalar1=1.0, scalar2=None,
                                        op0=Alu.not_equal)
                ces = psS.tile([1, 1], f32, tag="psmall")
                nc.tensor.matmul(out=ces[:], lhsT=ce2[:], rhs=ones_col[0:K, :],
                                 start=True, stop=True)
                nff = pool.tile([1, 2], f32, tag="nff")
                nc.gpsimd.tensor_copy(nff[:, 0:1], nf1[:])
                nc.gpsimd.tensor_copy(nff[:, 1:2], nf2[:])
                fl = pool.tile([1, 8], f32, tag="fl")
                nc.gpsimd.memset(fl[:], 0.0)
                nc.scalar.activation(fl[:, 0:1], fcaps[:], Act.Copy)
                nc.scalar.activation(fl[:, 1:2], ces[:], Act.Copy)
                nc.scalar.activation(fl[:, 2:3], fconv[:], Act.Copy)
                nc.vector.tensor_scalar(out=fl[:, 3:5], in0=nff[:],
                                        scalar1=float(W), scalar2=None,
                                        op0=Alu.is_gt)
                nc.sync.dma_start(out=flags[b], in_=fl[:])

    nc.compile()
    return nc


# ======================= host side =======================

IOU_THR = 0.5
SCORE_THR = 0.0


def _reference_numpy(preds_img):
    """Exact numpy clone of the jax reference for one image [5, H*W]."""
    s = preds_img[0].astype(np.float32)
    boxes = preds_img[1:5].astype(np.float32).T  # [N, 4]
    masked = np.where(s > SCORE_THR, s, -np.inf).astype(np.float32)
    order = np.argsort(-masked, kind="stable")[:K]
    top_vals = masked[order]
    top_boxes = boxes[order]
    valid = np.isfinite(top_vals)
    x1, y1, x2, y2 = (top_boxes[:, j] for j in range(4))
    lt_x = np.maximum(x1[:, None], x1[None, :])
    lt_y = np.maximum(y1[:, None], y1[None, :])
    rb_x = np.minimum(x2[:, None], x2[None, :])
    rb_y = np.minimum(y2[:, None], y2[None, :])
    wv = np.clip(rb_x - lt_x, 0.0, None).astype(np.float32)
    hv = np.clip(rb_y - lt_y, 0.0, None).astype(np.float32)
    inter = (wv * hv).astype(np.float32)
    area = ((x2 - x1) * (y2 - y1)).astype(np.float32)
    union = (area[:, None] + area[None, :] - inter).astype(np.float32)
    with np.errstate(divide="ignore", invalid="ignore"):
        iou = inter / union
    keep = valid.copy()
    idx = np.arange(K)
    for i in range(K):
        sup = (iou[i] > IOU_THR) & keep[i] & (idx > i)
        keep = keep & ~sup
    so = np.where(keep, top_vals, 0.0).astype(np.float32)
    bo = np.where(keep[:, None], top_boxes, 0.0).astype(np.float32)
    return np.concatenate([so[:, None], bo], axis=1)


_CACHE = {}


def kernel(preds):
    from concourse.bass_utils import run_bass_kernel_spmd

    preds = np.ascontiguousarray(np.asarray(preds), dtype=np.float32)
    B = preds.shape[0]
    pr = preds.reshape(B, 5, N)
    ncores = B // NIMG
    if "nc" not in _CACHE:
        _CACHE["nc"] = build_nc()
    in_maps = [
        {"preds": np.ascontiguousarray(pr[NIMG * i:NIMG * (i + 1)])}
        for i in range(ncores)
    ]
    res = run_bass_kernel_spmd(_CACHE["nc"], in_maps, core_ids=list(range(ncores)))
    outs = np.concatenate([r["out"] for r in res.results], axis=0)
    fl = np.concatenate([r["flags"] for r in res.results], axis=0)
    for img in range(B):
        if np.any(np.abs(fl[img]) > 0.5):
            outs[img] = _reference_numpy(pr[img])
    return outs.astype(np.float32)

